# revision 2
# baseline (speedup 1.0000x reference)
# Nadaraya-Watson kernel regression on TRN2, 8 NeuronCores, data-parallel over rows.
#
#   scores[n, k] = -0.5 * (w * (q[n] - keys[n, k]))^2
#   out[n] = sum_k softmax(scores[n, :])[k] * values[n, k]
#
# Sharding: rows (N=8192) split 8 ways -> 1024 rows/core; w replicated. No
# cross-core communication. Per core: 8 tiles of [128 rows x 4096 keys].
#
# Per-tile dataflow (memory-bound; DMA ~11.4us/tile vs ACT ~7.4us, DVE ~4.4us):
#   DMA  : kt <- keys[tile], vt <- values[tile]            (2 x 2MB, HWDGE)
#   ACT  : et = Square(kt + (-q))        = (k - q)^2       (bias AP [128,1])
#   ACT  : kt = Exp(c * et), se = sum    (c = -w^2/2, scale AP; fused accum)
#   DVE  : et = kt * vt, sp = sum        (tensor_tensor_reduce, fused accum)
# Tail: r = 1/se; ob = sp * r; PE-transpose [128,8]->[8,128]; one contiguous DMA.
#
# Softmax max-subtraction is skipped: scores <= 0 and each row's max score is
# ~0 (min |q-k| over 4096 N(0,2) samples is ~1e-4, w in (0,1)), so exp() never
# under/overflows and the result matches the max-subtracted form to fp32
# rounding.

import numpy as np

import concourse.bass as bass
import concourse.tile as tile
from concourse import bacc, mybir
from concourse.bass_utils import run_bass_kernel_spmd
from concourse.masks import make_identity

N, K = 8192, 4096
NCORES = 8
ROWS = N // NCORES  # 1024 rows per core
P = 128             # SBUF partitions
NTILES = ROWS // P  # 8 row-tiles per core

F32 = mybir.dt.float32

_cache = {}


def build_program():
    nc = bacc.Bacc(
        "TRN2",
        target_bir_lowering=False,
        debug=False,
        num_devices=NCORES,
    )

    q_d = nc.dram_tensor("queries", [ROWS], F32, kind="ExternalInput").ap()
    k_d = nc.dram_tensor("keys", [ROWS, K], F32, kind="ExternalInput").ap()
    v_d = nc.dram_tensor("values", [ROWS, K], F32, kind="ExternalInput").ap()
    w_d = nc.dram_tensor("w", [1], F32, kind="ExternalInput").ap()
    o_d = nc.dram_tensor("out", [ROWS], F32, kind="ExternalOutput").ap()

    with tile.TileContext(nc) as tc:
        with (
            tc.tile_pool(name="kpool", bufs=3) as kpool,
            tc.tile_pool(name="vpool", bufs=3) as vpool,
            tc.tile_pool(name="epool", bufs=2) as epool,
            tc.tile_pool(name="small", bufs=1) as small,
            tc.tile_pool(name="psum", bufs=1, space="PSUM") as psum,
        ):
            # ---- one-time setup (overlaps with first tile's loads) ----
            # w broadcast to all partitions, then c = -0.5 * w^2 per partition
            w_sb = small.tile([P, 1], F32)
            nc.gpsimd.dma_start(out=w_sb, in_=w_d.to_broadcast([P, 1]))
            c_sb = small.tile([P, 1], F32)
            nc.vector.tensor_scalar(
                c_sb, w_sb, w_sb, -0.5,
                mybir.AluOpType.mult, mybir.AluOpType.mult,
            )
            # queries laid out [128, NTILES]: q_sb[p, i] = q[i*128 + p]
            q_sb = small.tile([P, NTILES], F32)
            nc.gpsimd.dma_start(out=q_sb, in_=q_d.rearrange("(i p) -> p i", p=P))
            nq_sb = small.tile([P, NTILES], F32)  # -q (Square bias)
            nc.vector.tensor_scalar_mul(nq_sb, q_sb, -1.0)

            ident = small.tile([P, P], F32)
            make_identity(nc, ident)

            se = small.tile([P, NTILES], F32)  # sum of exp weights per row
            sp = small.tile([P, NTILES], F32)  # sum of exp * value per row

            # ---- main loop over row-tiles ----
            for i in range(NTILES):
                kt = kpool.tile([P, K], F32)
                nc.sync.dma_start(out=kt, in_=k_d[i * P:(i + 1) * P, :])
                vt = vpool.tile([P, K], F32)
                nc.sync.dma_start(out=vt, in_=v_d[i * P:(i + 1) * P, :])

                et = epool.tile([P, K], F32)
                # et = (k - q)^2
                nc.scalar.activation(
                    out=et, in_=kt,
                    func=mybir.ActivationFunctionType.Square,
                    bias=nq_sb[:, i:i + 1], scale=1.0,
                )
                # kt = exp(c * et), se[:, i] = sum_k kt   (kt buffer reused)
                nc.scalar.activation(
                    out=kt, in_=et,
                    func=mybir.ActivationFunctionType.Exp,
                    bias=0.0, scale=c_sb,
                    accum_out=se[:, i:i + 1],
                )
                # et = kt * vt, sp[:, i] = sum_k et       (et buffer reused)
                # (tensor_tensor_reduce faults on this HW path; scalar_tensor_tensor
                # with accum_out is the same single DVE pass)
                nc.vector.scalar_tensor_tensor(
                    out=et, in0=kt, scalar=1.0, in1=vt,
                    op0=mybir.AluOpType.mult, op1=mybir.AluOpType.mult,
                    accum_out=sp[:, i:i + 1],
                )

            # ---- tail: normalize and store ----
            r_sb = small.tile([P, NTILES], F32)
            nc.vector.reciprocal(out=r_sb, in_=se)
            ob = small.tile([P, NTILES], F32)
            nc.vector.tensor_mul(ob, sp, r_sb)

            # [128, 8] -> [8, 128] so the output DMA is 8 contiguous 512B rows
            obT_ps = psum.tile([NTILES, P], F32)
            nc.tensor.transpose(obT_ps, ob, ident)
            obT = small.tile([NTILES, P], F32)
            nc.vector.tensor_copy(out=obT, in_=obT_ps)
            nc.sync.dma_start(
                out=o_d.rearrange("(i p) -> i p", p=P), in_=obT
            )

    nc.compile()
    return nc


def get_program():
    if "nc" not in _cache:
        _cache["nc"] = build_program()
    return _cache["nc"]


def make_in_maps(queries, keys, values, w):
    queries = np.ascontiguousarray(np.asarray(queries, dtype=np.float32))
    keys = np.ascontiguousarray(np.asarray(keys, dtype=np.float32))
    values = np.ascontiguousarray(np.asarray(values, dtype=np.float32))
    w = np.ascontiguousarray(np.asarray(w, dtype=np.float32))
    return [
        {
            "queries": queries[c * ROWS:(c + 1) * ROWS],
            "keys": keys[c * ROWS:(c + 1) * ROWS],
            "values": values[c * ROWS:(c + 1) * ROWS],
            "w": w,
        }
        for c in range(NCORES)
    ]


def kernel(queries, keys, values, w):
    nc = get_program()
    res = run_bass_kernel_spmd(
        nc, make_in_maps(queries, keys, values, w), list(range(NCORES))
    ).results
    return np.concatenate([res[c]["out"] for c in range(NCORES)])


# revision 10
# speedup vs baseline: 1.2892x; 1.2892x over previous
# Nadaraya-Watson kernel regression on TRN2, 8 NeuronCores, data-parallel over rows.
#
#   scores[n, k] = -0.5 * (w * (q[n] - keys[n, k]))^2
#   out[n] = sum_k softmax(scores[n, :])[k] * values[n, k]
#
# Sharding: rows (N=8192) split 8 ways -> 1024 rows/core; w replicated. No
# cross-core communication. Per core: 8 tiles of [128 rows x 4096 keys].
#
# Per-tile dataflow (memory-bound; DMA ~11.4us/tile vs ACT ~7.4us, DVE ~4.4us):
#   DMA  : kt <- keys[tile], vt <- values[tile]            (2 x 2MB, HWDGE)
#   ACT  : et = Square(kt + (-q))        = (k - q)^2       (bias AP [128,1])
#   ACT  : kt = Exp(c * et), se = sum    (c = -w^2/2, scale AP; fused accum)
#   DVE  : et = kt * vt, sp = sum        (tensor_tensor_reduce, fused accum)
# Tail: r = 1/se; ob = sp * r; PE-transpose [128,8]->[8,128]; one contiguous DMA.
#
# Softmax max-subtraction is skipped: scores <= 0 and each row's max score is
# ~0 (min |q-k| over 4096 N(0,2) samples is ~1e-4, w in (0,1)), so exp() never
# under/overflows and the result matches the max-subtracted form to fp32
# rounding.

import numpy as np

import concourse.bass as bass
import concourse.tile as tile
from concourse import bacc, mybir
from concourse.bass_utils import run_bass_kernel_spmd

N, K = 8192, 4096
NCORES = 8
ROWS = N // NCORES  # 1024 rows per core
P = 128             # SBUF partitions
NTILES = ROWS // P  # 8 row-tiles per core

F32 = mybir.dt.float32

_cache = {}


def build_program():
    nc = bacc.Bacc(
        "TRN2",
        target_bir_lowering=False,
        debug=False,
        num_devices=NCORES,
    )

    q_d = nc.dram_tensor("queries", [ROWS], F32, kind="ExternalInput").ap()
    k_d = nc.dram_tensor("keys", [ROWS, K], F32, kind="ExternalInput").ap()
    v_d = nc.dram_tensor("values", [ROWS, K], F32, kind="ExternalInput").ap()
    w_d = nc.dram_tensor("w", [1], F32, kind="ExternalInput").ap()
    o_d = nc.dram_tensor("out", [ROWS], F32, kind="ExternalOutput").ap()

    LAST = NTILES - 1
    NCH = 4          # last tile: v-load + multiply-reduce chunked along K
    CK = K // NCH

    with tile.TileContext(nc) as tc:
        with (
            tc.tile_pool(name="kpool", bufs=3) as kpool,
            tc.tile_pool(name="vpool", bufs=3) as vpool,
            tc.tile_pool(name="epool", bufs=2) as epool,
            tc.tile_pool(name="small", bufs=1) as small,
            tc.tile_pool(name="cols", bufs=4) as cols,
        ):
            o_cols = o_d.rearrange("(i p) -> p i", p=P)  # [128, NTILES] view

            se = small.tile([P, NTILES], F32)   # sum of exp weights per row
            sp = small.tile([P, NTILES], F32)   # sum of exp * value per row
            spc = small.tile([P, 2 * NCH], F32)  # tiles 6/7 per-chunk partials

            # ---- DMA issue order on the sync (HWDGE) queue ----
            # k0 first so the load stream starts immediately; then the tiny
            # q/w loads (first Square is gated on -q, so they must not land
            # mid-stream); then the rest. k[LAST] is hoisted before v[LAST-1]
            # so the last tile's Square/Exp finish while v still streams, and
            # v[LAST] arrives in NCH chunks consumed chunkwise by the DVE pass.
            kts, vts = [], []
            for i in range(NTILES):
                kts.append(kpool.tile([P, K], F32, name=f"kt{i}", tag="kt"))
                vts.append(vpool.tile([P, K], F32, name=f"vt{i}", tag="vt"))

            def load_k(i):
                nc.sync.dma_start(out=kts[i], in_=k_d[i * P:(i + 1) * P, :])

            def load_v(i):
                nc.sync.dma_start(out=vts[i], in_=v_d[i * P:(i + 1) * P, :])

            load_k(0)

            # w broadcast to all partitions, then c = -0.5 * w^2 per partition
            w_sb = small.tile([P, 1], F32)
            nc.gpsimd.dma_start(out=w_sb, in_=w_d.to_broadcast([P, 1]))
            c_sb = small.tile([P, 1], F32)
            nc.vector.tensor_scalar(
                c_sb, w_sb, w_sb, -0.5,
                mybir.AluOpType.mult, mybir.AluOpType.mult,
            )
            # queries laid out [128, NTILES]: q_sb[p, i] = q[i*128 + p]
            q_sb = small.tile([P, NTILES], F32)
            nc.sync.dma_start(out=q_sb, in_=q_d.rearrange("(i p) -> p i", p=P))
            nq_sb = small.tile([P, NTILES], F32)  # -q (Square bias)
            nc.vector.tensor_scalar_mul(nq_sb, q_sb, -1.0)

            load_v(0)
            for i in range(1, LAST - 2):
                load_k(i)
                load_v(i)
            load_k(LAST - 2)          # k5
            load_k(LAST - 1)          # k6
            load_v(LAST - 2)          # v5
            load_k(LAST)              # k7
            for i in (LAST - 1, LAST):  # v6, v7 chunked
                for c in range(NCH):
                    ksl = slice(c * CK, (c + 1) * CK)
                    nc.sync.dma_start(
                        out=vts[i][:, ksl], in_=v_d[i * P:(i + 1) * P, ksl]
                    )

            # ---- compute ----
            def act_block(i, et):
                # et = (k - q)^2
                nc.scalar.activation(
                    out=et, in_=kts[i],
                    func=mybir.ActivationFunctionType.Square,
                    bias=nq_sb[:, i:i + 1], scale=1.0,
                )
                # kt = exp(c * et), se[:, i] = sum_k    (kt buffer reused)
                nc.scalar.activation(
                    out=kts[i], in_=et,
                    func=mybir.ActivationFunctionType.Exp,
                    bias=0.0, scale=c_sb,
                    accum_out=se[:, i:i + 1],
                )

            def mul_reduce(i, et, ksl, sp_out):
                # et = e * v, sp_out = sum_k et         (et buffer reused)
                # (tensor_tensor_reduce faults on this HW path; scalar_tensor_tensor
                # with accum_out is the same single DVE pass)
                nc.vector.scalar_tensor_tensor(
                    out=et[:, ksl], in0=kts[i][:, ksl], scalar=1.0,
                    in1=vts[i][:, ksl],
                    op0=mybir.AluOpType.mult, op1=mybir.AluOpType.mult,
                    accum_out=sp_out,
                )

            def store_row_tile(i):
                # out col i = sp[:, i] / se[:, i]; 512B partition-strided DMA.
                # Mid-stream stores go on the SWDGE (gpsimd) ring: the big
                # loads stream on the sync HWDGE ring in FIFO order, so a
                # dependent store there would head-of-line block the prefetch
                # stream. The final store uses the (drained) sync ring for its
                # lower dispatch latency.
                r_col = cols.tile([P, 1], F32, name=f"r_col{i}", tag="r_col")
                nc.vector.reciprocal(out=r_col, in_=se[:, i:i + 1])
                o_col = cols.tile([P, 1], F32, name=f"o_col{i}", tag="o_col")
                nc.vector.tensor_mul(o_col, sp[:, i:i + 1], r_col)
                eng = nc.sync if i == LAST else nc.gpsimd
                eng.dma_start(out=o_cols[:, i:i + 1], in_=o_col)

            for i in range(NTILES):
                et = epool.tile([P, K], F32, name=f"et{i}", tag="et")
                act_block(i, et)
                if i < LAST - 1:
                    mul_reduce(i, et, slice(0, K), sp[:, i:i + 1])
                else:
                    pc = (i - (LAST - 1)) * NCH
                    for c in range(NCH):
                        mul_reduce(i, et, slice(c * CK, (c + 1) * CK),
                                   spc[:, pc + c:pc + c + 1])
                    nc.vector.tensor_reduce(
                        out=sp[:, i:i + 1], in_=spc[:, pc:pc + NCH],
                        axis=mybir.AxisListType.X, op=mybir.AluOpType.add,
                    )
                store_row_tile(i)

    nc.compile()
    return nc


def get_program():
    if "nc" not in _cache:
        _cache["nc"] = build_program()
    return _cache["nc"]


def make_in_maps(queries, keys, values, w):
    queries = np.ascontiguousarray(np.asarray(queries, dtype=np.float32))
    keys = np.ascontiguousarray(np.asarray(keys, dtype=np.float32))
    values = np.ascontiguousarray(np.asarray(values, dtype=np.float32))
    w = np.ascontiguousarray(np.asarray(w, dtype=np.float32))
    return [
        {
            "queries": queries[c * ROWS:(c + 1) * ROWS],
            "keys": keys[c * ROWS:(c + 1) * ROWS],
            "values": values[c * ROWS:(c + 1) * ROWS],
            "w": w,
        }
        for c in range(NCORES)
    ]


def kernel(queries, keys, values, w):
    nc = get_program()
    res = run_bass_kernel_spmd(
        nc, make_in_maps(queries, keys, values, w), list(range(NCORES))
    ).results
    return np.concatenate([res[c]["out"] for c in range(NCORES)])


# revision 11
# speedup vs baseline: 561.5889x; 435.5946x over previous
# Nadaraya-Watson kernel regression on TRN2, 8 NeuronCores, data-parallel over rows.
#
#   scores[n, k] = -0.5 * (w * (q[n] - keys[n, k]))^2
#   out[n] = sum_k softmax(scores[n, :])[k] * values[n, k]
#
# Sharding: rows (N=8192) split 8 ways -> 1024 rows/core; w replicated. No
# cross-core communication. Per core: 8 tiles of [128 rows x 4096 keys].
#
# Per-tile dataflow (memory-bound; DMA ~11.4us/tile vs ACT ~7.4us, DVE ~4.4us):
#   DMA  : kt <- keys[tile], vt <- values[tile]            (2 x 2MB, HWDGE)
#   ACT  : et = Square(kt + (-q))        = (k - q)^2       (bias AP [128,1])
#   ACT  : kt = Exp(c * et), se = sum    (c = -w^2/2, scale AP; fused accum)
#   DVE  : et = kt * vt, sp = sum        (tensor_tensor_reduce, fused accum)
# Tail: r = 1/se; ob = sp * r; PE-transpose [128,8]->[8,128]; one contiguous DMA.
#
# Softmax max-subtraction is skipped: scores <= 0 and each row's max score is
# ~0 (min |q-k| over 4096 N(0,2) samples is ~1e-4, w in (0,1)), so exp() never
# under/overflows and the result matches the max-subtracted form to fp32
# rounding.

import numpy as np

import concourse.bass as bass
import concourse.tile as tile
from concourse import bacc, mybir
from concourse.bass_utils import run_bass_kernel_spmd

N, K = 8192, 4096
NCORES = 8
ROWS = N // NCORES  # 1024 rows per core
P = 128             # SBUF partitions
NTILES = ROWS // P  # 8 row-tiles per core

F32 = mybir.dt.float32

_cache = {}


def build_program(reps=1):
    nc = bacc.Bacc(
        "TRN2",
        target_bir_lowering=False,
        debug=False,
        num_devices=NCORES,
    )

    q_d = nc.dram_tensor("queries", [ROWS], F32, kind="ExternalInput").ap()
    k_d = nc.dram_tensor("keys", [ROWS, K], F32, kind="ExternalInput").ap()
    v_d = nc.dram_tensor("values", [ROWS, K], F32, kind="ExternalInput").ap()
    w_d = nc.dram_tensor("w", [1], F32, kind="ExternalInput").ap()
    o_d = nc.dram_tensor("out", [ROWS], F32, kind="ExternalOutput").ap()

    LAST = NTILES - 1
    NCH = 4          # last tile: v-load + multiply-reduce chunked along K
    CK = K // NCH

    with tile.TileContext(nc) as tc:
        with (
            tc.tile_pool(name="kpool", bufs=3) as kpool,
            tc.tile_pool(name="vpool", bufs=3) as vpool,
            tc.tile_pool(name="epool", bufs=2) as epool,
            tc.tile_pool(name="small", bufs=1) as small,
            tc.tile_pool(name="cols", bufs=4) as cols,
        ):
            o_cols = o_d.rearrange("(i p) -> p i", p=P)  # [128, NTILES] view

            se = small.tile([P, NTILES], F32)   # sum of exp weights per row
            sp = small.tile([P, NTILES], F32)   # sum of exp * value per row
            spc = small.tile([P, 2 * NCH], F32)  # tiles 6/7 per-chunk partials

            # ---- DMA issue order on the sync (HWDGE) queue ----
            # k0 first so the load stream starts immediately; then the tiny
            # q/w loads (first Square is gated on -q, so they must not land
            # mid-stream); then the rest. k[LAST] is hoisted before v[LAST-1]
            # so the last tile's Square/Exp finish while v still streams, and
            # v[LAST] arrives in NCH chunks consumed chunkwise by the DVE pass.
            kts, vts = [], []

            def alloc_kv(r):
                kts.clear(); vts.clear()
                for i in range(NTILES):
                    kts.append(kpool.tile([P, K], F32, name=f"kt{i}_r{r}", tag="kt"))
                    vts.append(vpool.tile([P, K], F32, name=f"vt{i}_r{r}", tag="vt"))

            def load_k(i):
                nc.sync.dma_start(out=kts[i], in_=k_d[i * P:(i + 1) * P, :])

            def load_v(i):
                nc.sync.dma_start(out=vts[i], in_=v_d[i * P:(i + 1) * P, :])

            alloc_kv(0)
            load_k(0)

            # w broadcast to all partitions, then c = -0.5 * w^2 per partition
            w_sb = small.tile([P, 1], F32)
            nc.gpsimd.dma_start(out=w_sb, in_=w_d.to_broadcast([P, 1]))
            c_sb = small.tile([P, 1], F32)
            nc.vector.tensor_scalar(
                c_sb, w_sb, w_sb, -0.5,
                mybir.AluOpType.mult, mybir.AluOpType.mult,
            )
            # queries laid out [128, NTILES]: q_sb[p, i] = q[i*128 + p]
            q_sb = small.tile([P, NTILES], F32)
            nc.sync.dma_start(out=q_sb, in_=q_d.rearrange("(i p) -> p i", p=P))
            nq_sb = small.tile([P, NTILES], F32)  # -q (Square bias)
            nc.vector.tensor_scalar_mul(nq_sb, q_sb, -1.0)

            def load_rest():
                load_v(0)
                for i in range(1, LAST - 2):
                    load_k(i)
                    load_v(i)
                load_k(LAST - 2)          # k5
                load_k(LAST - 1)          # k6
                load_v(LAST - 2)          # v5
                load_k(LAST)              # k7
                for i in (LAST - 1, LAST):  # v6, v7 chunked
                    for c in range(NCH):
                        ksl = slice(c * CK, (c + 1) * CK)
                        nc.sync.dma_start(
                            out=vts[i][:, ksl], in_=v_d[i * P:(i + 1) * P, ksl]
                        )

            # ---- compute ----
            def act_block(i, et):
                # et = (k - q)^2
                nc.scalar.activation(
                    out=et, in_=kts[i],
                    func=mybir.ActivationFunctionType.Square,
                    bias=nq_sb[:, i:i + 1], scale=1.0,
                )
                # kt = exp(c * et), se[:, i] = sum_k    (kt buffer reused)
                nc.scalar.activation(
                    out=kts[i], in_=et,
                    func=mybir.ActivationFunctionType.Exp,
                    bias=0.0, scale=c_sb,
                    accum_out=se[:, i:i + 1],
                )

            def mul_reduce(i, et, ksl, sp_out):
                # et = e * v, sp_out = sum_k et         (et buffer reused)
                # (tensor_tensor_reduce faults on this HW path; scalar_tensor_tensor
                # with accum_out is the same single DVE pass)
                nc.vector.scalar_tensor_tensor(
                    out=et[:, ksl], in0=kts[i][:, ksl], scalar=1.0,
                    in1=vts[i][:, ksl],
                    op0=mybir.AluOpType.mult, op1=mybir.AluOpType.mult,
                    accum_out=sp_out,
                )

            def store_row_tile(i, last_on_sync):
                # out col i = sp[:, i] / se[:, i]; 512B partition-strided DMA.
                # Mid-stream stores go on the SWDGE (gpsimd) ring: the big
                # loads stream on the sync HWDGE ring in FIFO order, so a
                # dependent store there would head-of-line block the prefetch
                # stream. The final store uses the (drained) sync ring for its
                # lower dispatch latency.
                r_col = cols.tile([P, 1], F32, name=f"r_col{i}_{nc.next_id()}", tag="r_col")
                nc.vector.reciprocal(out=r_col, in_=se[:, i:i + 1])
                o_col = cols.tile([P, 1], F32, name=f"o_col{i}_{nc.next_id()}", tag="o_col")
                nc.vector.tensor_mul(o_col, sp[:, i:i + 1], r_col)
                eng = nc.sync if (i == LAST and last_on_sync) else nc.gpsimd
                eng.dma_start(out=o_cols[:, i:i + 1], in_=o_col)

            def compute_all(r, last_on_sync):
                for i in range(NTILES):
                    et = epool.tile([P, K], F32, name=f"et{i}_r{r}", tag="et")
                    act_block(i, et)
                    if i < LAST - 1:
                        mul_reduce(i, et, slice(0, K), sp[:, i:i + 1])
                    else:
                        pc = (i - (LAST - 1)) * NCH
                        for c in range(NCH):
                            mul_reduce(i, et, slice(c * CK, (c + 1) * CK),
                                       spc[:, pc + c:pc + c + 1])
                        nc.vector.tensor_reduce(
                            out=sp[:, i:i + 1], in_=spc[:, pc:pc + NCH],
                            axis=mybir.AxisListType.X, op=mybir.AluOpType.add,
                        )
                    store_row_tile(i, last_on_sync)

            load_rest()
            compute_all(0, last_on_sync=(reps == 1))
            # extra reps (timing-only variants): identical work, re-reading
            # the same inputs and rewriting the same outputs
            for r in range(1, reps):
                alloc_kv(r)
                for i in range(NTILES):
                    load_k(i)
                    load_v(i)
                compute_all(r, last_on_sync=False)

    nc.compile()
    return nc


def get_program():
    if "nc" not in _cache:
        _cache["nc"] = build_program()
    return _cache["nc"]


def make_in_maps(queries, keys, values, w):
    queries = np.ascontiguousarray(np.asarray(queries, dtype=np.float32))
    keys = np.ascontiguousarray(np.asarray(keys, dtype=np.float32))
    values = np.ascontiguousarray(np.asarray(values, dtype=np.float32))
    w = np.ascontiguousarray(np.asarray(w, dtype=np.float32))
    return [
        {
            "queries": queries[c * ROWS:(c + 1) * ROWS],
            "keys": keys[c * ROWS:(c + 1) * ROWS],
            "values": values[c * ROWS:(c + 1) * ROWS],
            "w": w,
        }
        for c in range(NCORES)
    ]


def kernel(queries, keys, values, w):
    nc = get_program()
    res = run_bass_kernel_spmd(
        nc, make_in_maps(queries, keys, values, w), list(range(NCORES))
    ).results
    return np.concatenate([res[c]["out"] for c in range(NCORES)])


# revision 12
# speedup vs baseline: 879.2202x; 1.5656x over previous
# Nadaraya-Watson kernel regression on TRN2, 8 NeuronCores, data-parallel over rows.
#
#   scores[n, k] = -0.5 * (w * (q[n] - keys[n, k]))^2
#   out[n] = sum_k softmax(scores[n, :])[k] * values[n, k]
#
# Sharding: rows (N=8192) split 8 ways -> 1024 rows/core; w replicated. No
# cross-core communication. Per core: 8 tiles of [128 rows x 4096 keys].
#
# Per-tile dataflow (memory-bound; DMA ~11.4us/tile vs ACT ~7.4us, DVE ~4.4us):
#   DMA  : kt <- keys[tile], vt <- values[tile]            (2 x 2MB, HWDGE)
#   ACT  : et = Square(kt + (-q))        = (k - q)^2       (bias AP [128,1])
#   ACT  : kt = Exp(c * et), se = sum    (c = -w^2/2, scale AP; fused accum)
#   DVE  : et = kt * vt, sp = sum        (tensor_tensor_reduce, fused accum)
# Tail: r = 1/se; ob = sp * r; PE-transpose [128,8]->[8,128]; one contiguous DMA.
#
# Softmax max-subtraction is skipped: scores <= 0 and each row's max score is
# ~0 (min |q-k| over 4096 N(0,2) samples is ~1e-4, w in (0,1)), so exp() never
# under/overflows and the result matches the max-subtracted form to fp32
# rounding.

import numpy as np

import concourse.bass as bass
import concourse.tile as tile
from concourse import bacc, mybir
from concourse.bass_utils import run_bass_kernel_spmd
from concourse.masks import make_identity

N, K = 8192, 4096
NCORES = 8
ROWS = N // NCORES  # 1024 rows per core
P = 128             # SBUF partitions
NTILES = ROWS // P  # 8 row-tiles per core

F32 = mybir.dt.float32

_cache = {}


def build_program(reps=1):
    nc = bacc.Bacc(
        "TRN2",
        target_bir_lowering=False,
        debug=False,
        num_devices=NCORES,
    )

    q_d = nc.dram_tensor("queries", [ROWS], F32, kind="ExternalInput").ap()
    k_d = nc.dram_tensor("keys", [ROWS, K], F32, kind="ExternalInput").ap()
    v_d = nc.dram_tensor("values", [ROWS, K], F32, kind="ExternalInput").ap()
    w_d = nc.dram_tensor("w", [1], F32, kind="ExternalInput").ap()
    o_d = nc.dram_tensor("out", [ROWS], F32, kind="ExternalOutput").ap()

    LAST = NTILES - 1
    NCH = 4          # last tile: v-load + multiply-reduce chunked along K
    CK = K // NCH

    with tile.TileContext(nc) as tc:
        with (
            tc.tile_pool(name="kpool", bufs=3) as kpool,
            tc.tile_pool(name="vpool", bufs=3) as vpool,
            tc.tile_pool(name="epool", bufs=2) as epool,
            tc.tile_pool(name="small", bufs=1) as small,
            tc.tile_pool(name="cols", bufs=2) as cols,
            tc.tile_pool(name="psum", bufs=2, space="PSUM") as psum,
        ):
            o_rows = o_d.rearrange("(i p) -> i p", p=P)  # [NTILES, 128] view
            ident = small.tile([P, P], F32)
            make_identity(nc, ident)

            se = small.tile([P, NTILES], F32)   # sum of exp weights per row
            sp = small.tile([P, NTILES], F32)   # sum of exp * value per row
            spc = small.tile([P, 2 * NCH], F32)  # tiles 6/7 per-chunk partials

            # ---- DMA issue order on the sync (HWDGE) queue ----
            # k0 first so the load stream starts immediately; then the tiny
            # q/w loads (first Square is gated on -q, so they must not land
            # mid-stream); then the rest. k[LAST] is hoisted before v[LAST-1]
            # so the last tile's Square/Exp finish while v still streams, and
            # v[LAST] arrives in NCH chunks consumed chunkwise by the DVE pass.
            kts, vts = [], []

            def alloc_kv(r):
                kts.clear(); vts.clear()
                for i in range(NTILES):
                    kts.append(kpool.tile([P, K], F32, name=f"kt{i}_r{r}", tag="kt"))
                    vts.append(vpool.tile([P, K], F32, name=f"vt{i}_r{r}", tag="vt"))

            def load_k(i):
                nc.sync.dma_start(out=kts[i], in_=k_d[i * P:(i + 1) * P, :])

            def load_v(i):
                nc.sync.dma_start(out=vts[i], in_=v_d[i * P:(i + 1) * P, :])

            alloc_kv(0)
            load_k(0)

            # w broadcast to all partitions, then c = -0.5 * w^2 per partition
            w_sb = small.tile([P, 1], F32)
            nc.gpsimd.dma_start(out=w_sb, in_=w_d.to_broadcast([P, 1]))
            c_sb = small.tile([P, 1], F32)
            nc.vector.tensor_scalar(
                c_sb, w_sb, w_sb, -0.5,
                mybir.AluOpType.mult, mybir.AluOpType.mult,
            )
            # queries laid out [128, NTILES]: q_sb[p, i] = q[i*128 + p]
            q_sb = small.tile([P, NTILES], F32)
            nc.sync.dma_start(out=q_sb, in_=q_d.rearrange("(i p) -> p i", p=P))
            nq_sb = small.tile([P, NTILES], F32)  # -q (Square bias)
            nc.vector.tensor_scalar_mul(nq_sb, q_sb, -1.0)

            def load_rest():
                load_v(0)
                for i in range(1, LAST - 2):
                    load_k(i)
                    load_v(i)
                load_k(LAST - 2)          # k5
                load_k(LAST - 1)          # k6
                load_v(LAST - 2)          # v5
                load_k(LAST)              # k7
                for i in (LAST - 1, LAST):  # v6, v7 chunked
                    for c in range(NCH):
                        ksl = slice(c * CK, (c + 1) * CK)
                        nc.sync.dma_start(
                            out=vts[i][:, ksl], in_=v_d[i * P:(i + 1) * P, ksl]
                        )

            # ---- compute ----
            def act_block(i, et):
                # et = (k - q)^2
                nc.scalar.activation(
                    out=et, in_=kts[i],
                    func=mybir.ActivationFunctionType.Square,
                    bias=nq_sb[:, i:i + 1], scale=1.0,
                )
                # kt = exp(c * et), se[:, i] = sum_k    (kt buffer reused)
                nc.scalar.activation(
                    out=kts[i], in_=et,
                    func=mybir.ActivationFunctionType.Exp,
                    bias=0.0, scale=c_sb,
                    accum_out=se[:, i:i + 1],
                )

            def mul_reduce(i, et, ksl, sp_out):
                # et = e * v, sp_out = sum_k et         (et buffer reused)
                # (tensor_tensor_reduce faults on this HW path; scalar_tensor_tensor
                # with accum_out is the same single DVE pass)
                nc.vector.scalar_tensor_tensor(
                    out=et[:, ksl], in0=kts[i][:, ksl], scalar=1.0,
                    in1=vts[i][:, ksl],
                    op0=mybir.AluOpType.mult, op1=mybir.AluOpType.mult,
                    accum_out=sp_out,
                )

            def store_all(r, on_sync):
                # out = sp / se for all 8 row-tiles at once, PE-transposed to
                # [8, 128] so the store is 8 contiguous 512B rows. A scattered
                # [128, 1]-per-tile store (128 x 4B descriptors) costs ~6us
                # each on HW; this whole chain is ~2us. Mid-stream (timing
                # reps) the store goes on the SWDGE ring so it cannot
                # head-of-line block the sync-ring prefetch stream.
                rall = cols.tile([P, NTILES], F32, name=f"rall_r{r}", tag="rall")
                nc.vector.reciprocal(out=rall, in_=se)
                ob = cols.tile([P, NTILES], F32, name=f"ob_r{r}", tag="ob")
                nc.vector.tensor_mul(ob, sp, rall)
                obT_ps = psum.tile([NTILES, P], F32, name=f"obTp_r{r}", tag="obTp")
                nc.tensor.transpose(obT_ps, ob, ident)
                obT = cols.tile([NTILES, P], F32, name=f"obT_r{r}", tag="obT")
                nc.vector.tensor_copy(out=obT, in_=obT_ps)
                eng = nc.sync if on_sync else nc.gpsimd
                eng.dma_start(out=o_rows, in_=obT)

            def compute_all(r, last_on_sync):
                for i in range(NTILES):
                    et = epool.tile([P, K], F32, name=f"et{i}_r{r}", tag="et")
                    act_block(i, et)
                    if i < LAST - 1:
                        mul_reduce(i, et, slice(0, K), sp[:, i:i + 1])
                    else:
                        pc = (i - (LAST - 1)) * NCH
                        for c in range(NCH):
                            mul_reduce(i, et, slice(c * CK, (c + 1) * CK),
                                       spc[:, pc + c:pc + c + 1])
                        nc.vector.tensor_reduce(
                            out=sp[:, i:i + 1], in_=spc[:, pc:pc + NCH],
                            axis=mybir.AxisListType.X, op=mybir.AluOpType.add,
                        )
                store_all(r, on_sync=last_on_sync)

            load_rest()
            compute_all(0, last_on_sync=(reps == 1))
            # extra reps (timing-only variants): identical work, re-reading
            # the same inputs and rewriting the same outputs
            for r in range(1, reps):
                alloc_kv(r)
                for i in range(NTILES):
                    load_k(i)
                    load_v(i)
                compute_all(r, last_on_sync=False)

    nc.compile()
    return nc


def get_program():
    if "nc" not in _cache:
        _cache["nc"] = build_program()
    return _cache["nc"]


def make_in_maps(queries, keys, values, w):
    queries = np.ascontiguousarray(np.asarray(queries, dtype=np.float32))
    keys = np.ascontiguousarray(np.asarray(keys, dtype=np.float32))
    values = np.ascontiguousarray(np.asarray(values, dtype=np.float32))
    w = np.ascontiguousarray(np.asarray(w, dtype=np.float32))
    return [
        {
            "queries": queries[c * ROWS:(c + 1) * ROWS],
            "keys": keys[c * ROWS:(c + 1) * ROWS],
            "values": values[c * ROWS:(c + 1) * ROWS],
            "w": w,
        }
        for c in range(NCORES)
    ]


def kernel(queries, keys, values, w):
    nc = get_program()
    res = run_bass_kernel_spmd(
        nc, make_in_maps(queries, keys, values, w), list(range(NCORES))
    ).results
    return np.concatenate([res[c]["out"] for c in range(NCORES)])


# revision 13
# speedup vs baseline: 967.9711x; 1.1009x over previous
# Nadaraya-Watson kernel regression on TRN2, 8 NeuronCores, data-parallel over rows.
#
#   scores[n, k] = -0.5 * (w * (q[n] - keys[n, k]))^2
#   out[n] = sum_k softmax(scores[n, :])[k] * values[n, k]
#
# Sharding: rows (N=8192) split 8 ways -> 1024 rows/core; w replicated. No
# cross-core communication. Per core: 8 tiles of [128 rows x 4096 keys].
#
# Per-tile dataflow (memory-bound; DMA ~11.4us/tile vs ACT ~7.4us, DVE ~4.4us):
#   DMA  : kt <- keys[tile], vt <- values[tile]            (2 x 2MB, HWDGE)
#   ACT  : et = Square(kt + (-q))        = (k - q)^2       (bias AP [128,1])
#   ACT  : kt = Exp(c * et), se = sum    (c = -w^2/2, scale AP; fused accum)
#   DVE  : et = kt * vt, sp = sum        (scalar_tensor_tensor, fused accum)
# Tail: r = 1/se; ob = sp * r; PE-transpose [128,8]->[8,128]; one contiguous
# 4KB store (a per-tile [128,1] store is 128 x 4B descriptors, ~6us each on
# HW). The last row-tile's v-load + DVE pass are chunked along K and k[LAST]
# is hoisted ahead in the stream so the final serial chain after the last
# input byte is ~1 chunk of DVE work instead of a full Square+Exp+mul-reduce.
#
# Softmax max-subtraction is skipped: scores <= 0 and each row's max score is
# ~0 (min |q-k| over 4096 N(0,2) samples is ~1e-4, w in (0,1)), so exp() never
# under/overflows and the result matches the max-subtracted form to fp32
# rounding.

import numpy as np

import concourse.bass as bass
import concourse.tile as tile
from concourse import bacc, mybir
from concourse.bass_utils import run_bass_kernel_spmd
from concourse.masks import make_identity

N, K = 8192, 4096
NCORES = 8
ROWS = N // NCORES  # 1024 rows per core
P = 128             # SBUF partitions
NTILES = ROWS // P  # 8 row-tiles per core

F32 = mybir.dt.float32

_cache = {}


def build_program(reps=1):
    nc = bacc.Bacc(
        "TRN2",
        target_bir_lowering=False,
        debug=False,
        num_devices=NCORES,
    )

    q_d = nc.dram_tensor("queries", [ROWS], F32, kind="ExternalInput").ap()
    k_d = nc.dram_tensor("keys", [ROWS, K], F32, kind="ExternalInput").ap()
    v_d = nc.dram_tensor("values", [ROWS, K], F32, kind="ExternalInput").ap()
    w_d = nc.dram_tensor("w", [1], F32, kind="ExternalInput").ap()
    o_d = nc.dram_tensor("out", [ROWS], F32, kind="ExternalOutput").ap()

    LAST = NTILES - 1
    NCH = 4          # last tile: v-load + multiply-reduce chunked along K
    CK = K // NCH

    with tile.TileContext(nc) as tc:
        with (
            tc.tile_pool(name="kpool", bufs=3) as kpool,
            tc.tile_pool(name="vpool", bufs=3) as vpool,
            tc.tile_pool(name="epool", bufs=2) as epool,
            tc.tile_pool(name="small", bufs=1) as small,
            tc.tile_pool(name="cols", bufs=2) as cols,
            tc.tile_pool(name="psum", bufs=2, space="PSUM") as psum,
        ):
            o_rows = o_d.rearrange("(i p) -> i p", p=P)  # [NTILES, 128] view
            ident = small.tile([P, P], F32)
            make_identity(nc, ident)

            se = small.tile([P, NTILES], F32)   # sum of exp weights per row
            sp = small.tile([P, NTILES], F32)   # sum of exp * value per row
            spc = small.tile([P, 2 * NCH], F32)  # tiles 6/7 per-chunk partials

            # ---- DMA issue order on the sync (HWDGE) queue ----
            # k0 first so the load stream starts immediately; then the tiny
            # q/w loads (first Square is gated on -q, so they must not land
            # mid-stream); then the rest. k[LAST] is hoisted before v[LAST-1]
            # so the last tile's Square/Exp finish while v still streams, and
            # v[LAST] arrives in NCH chunks consumed chunkwise by the DVE pass.
            kts, vts = [], []

            def alloc_kv(r):
                kts.clear(); vts.clear()
                for i in range(NTILES):
                    kts.append(kpool.tile([P, K], F32, name=f"kt{i}_r{r}", tag="kt"))
                    vts.append(vpool.tile([P, K], F32, name=f"vt{i}_r{r}", tag="vt"))

            def load_k(i):
                nc.sync.dma_start(out=kts[i], in_=k_d[i * P:(i + 1) * P, :])

            def load_v(i):
                nc.sync.dma_start(out=vts[i], in_=v_d[i * P:(i + 1) * P, :])

            alloc_kv(0)
            load_k(0)

            # w broadcast to all partitions, then c = -0.5 * w^2 per partition
            w_sb = small.tile([P, 1], F32)
            nc.gpsimd.dma_start(out=w_sb, in_=w_d.to_broadcast([P, 1]))
            c_sb = small.tile([P, 1], F32)
            nc.vector.tensor_scalar(
                c_sb, w_sb, w_sb, -0.5,
                mybir.AluOpType.mult, mybir.AluOpType.mult,
            )
            # queries laid out [128, NTILES]: q_sb[p, i] = q[i*128 + p]
            q_sb = small.tile([P, NTILES], F32)
            nc.sync.dma_start(out=q_sb, in_=q_d.rearrange("(i p) -> p i", p=P))
            nq_sb = small.tile([P, NTILES], F32)  # -q (Square bias)
            nc.vector.tensor_scalar_mul(nq_sb, q_sb, -1.0)

            def load_rest():
                load_v(0)
                for i in range(1, LAST - 2):
                    load_k(i)
                    load_v(i)
                load_k(LAST - 2)          # k5
                load_k(LAST - 1)          # k6
                load_v(LAST - 2)          # v5
                load_k(LAST)              # k7
                for i in (LAST - 1, LAST):  # v6, v7 chunked
                    for c in range(NCH):
                        ksl = slice(c * CK, (c + 1) * CK)
                        nc.sync.dma_start(
                            out=vts[i][:, ksl], in_=v_d[i * P:(i + 1) * P, ksl]
                        )

            # ---- compute ----
            def act_block(i, et):
                # et = (k - q)^2
                nc.scalar.activation(
                    out=et, in_=kts[i],
                    func=mybir.ActivationFunctionType.Square,
                    bias=nq_sb[:, i:i + 1], scale=1.0,
                )
                # kt = exp(c * et), se[:, i] = sum_k    (kt buffer reused)
                nc.scalar.activation(
                    out=kts[i], in_=et,
                    func=mybir.ActivationFunctionType.Exp,
                    bias=0.0, scale=c_sb,
                    accum_out=se[:, i:i + 1],
                )

            def mul_reduce(i, et, ksl, sp_out):
                # et = e * v, sp_out = sum_k et         (et buffer reused)
                # (tensor_tensor_reduce faults on this HW path; scalar_tensor_tensor
                # with accum_out is the same single DVE pass)
                nc.vector.scalar_tensor_tensor(
                    out=et[:, ksl], in0=kts[i][:, ksl], scalar=1.0,
                    in1=vts[i][:, ksl],
                    op0=mybir.AluOpType.mult, op1=mybir.AluOpType.mult,
                    accum_out=sp_out,
                )

            def store_all(r, on_sync):
                # out = sp / se for all 8 row-tiles at once, PE-transposed to
                # [8, 128] so the store is 8 contiguous 512B rows. A scattered
                # [128, 1]-per-tile store (128 x 4B descriptors) costs ~6us
                # each on HW; this whole chain is ~2us. Mid-stream (timing
                # reps) the store goes on the SWDGE ring so it cannot
                # head-of-line block the sync-ring prefetch stream.
                rall = cols.tile([P, NTILES], F32, name=f"rall_r{r}", tag="rall")
                nc.vector.reciprocal(out=rall, in_=se)
                ob = cols.tile([P, NTILES], F32, name=f"ob_r{r}", tag="ob")
                nc.vector.tensor_mul(ob, sp, rall)
                obT_ps = psum.tile([NTILES, P], F32, name=f"obTp_r{r}", tag="obTp")
                nc.tensor.transpose(obT_ps, ob, ident)
                obT = cols.tile([NTILES, P], F32, name=f"obT_r{r}", tag="obT")
                nc.vector.tensor_copy(out=obT, in_=obT_ps)
                eng = nc.sync if on_sync else nc.gpsimd
                eng.dma_start(out=o_rows, in_=obT)

            def compute_all(r, last_on_sync):
                for i in range(NTILES):
                    et = epool.tile([P, K], F32, name=f"et{i}_r{r}", tag="et")
                    act_block(i, et)
                    if i < LAST - 1:
                        mul_reduce(i, et, slice(0, K), sp[:, i:i + 1])
                    else:
                        pc = (i - (LAST - 1)) * NCH
                        for c in range(NCH):
                            mul_reduce(i, et, slice(c * CK, (c + 1) * CK),
                                       spc[:, pc + c:pc + c + 1])
                        nc.vector.tensor_reduce(
                            out=sp[:, i:i + 1], in_=spc[:, pc:pc + NCH],
                            axis=mybir.AxisListType.X, op=mybir.AluOpType.add,
                        )
                store_all(r, on_sync=last_on_sync)

            load_rest()
            compute_all(0, last_on_sync=(reps == 1))
            # extra reps (timing-only variants): identical work, re-reading
            # the same inputs and rewriting the same outputs
            for r in range(1, reps):
                alloc_kv(r)
                for i in range(NTILES):
                    load_k(i)
                    load_v(i)
                compute_all(r, last_on_sync=False)

    nc.compile()
    return nc


def get_program():
    if "nc" not in _cache:
        _cache["nc"] = build_program()
    return _cache["nc"]


def make_in_maps(queries, keys, values, w):
    queries = np.ascontiguousarray(np.asarray(queries, dtype=np.float32))
    keys = np.ascontiguousarray(np.asarray(keys, dtype=np.float32))
    values = np.ascontiguousarray(np.asarray(values, dtype=np.float32))
    w = np.ascontiguousarray(np.asarray(w, dtype=np.float32))
    return [
        {
            "queries": queries[c * ROWS:(c + 1) * ROWS],
            "keys": keys[c * ROWS:(c + 1) * ROWS],
            "values": values[c * ROWS:(c + 1) * ROWS],
            "w": w,
        }
        for c in range(NCORES)
    ]


def kernel(queries, keys, values, w):
    nc = get_program()
    res = run_bass_kernel_spmd(
        nc, make_in_maps(queries, keys, values, w), list(range(NCORES))
    ).results
    return np.concatenate([res[c]["out"] for c in range(NCORES)])
